# revision 24
# baseline (speedup 1.0000x reference)
"""Trainium2 Bass kernel for nn_AttentionLayer (S=21, B=8192, H=1024).

Math (per batch element b):
    wc    = c[b] @ W                       # [H]
    vh_s  = allh[s,b] @ V                  # [H] per s
    e_sb  = tanh(wc + vh_s + bias[s])      # [S, H]
    sc_sb = e_sb . Wt                      # [S]
    alpha = softmax_s(sc)                  # [S]
    y[b]  = sum_s alpha_s * allh[s,b]      # [H]

Strategy: pure data parallel over batch across 8 NeuronCores (1024 batch
each).  Per core, the V/W matmuls run in a transposed layout (H on the
partition dim).  allh is loaded fp32->bf16 (SWDGE cast DMA) in natural
layout, transposed on-chip via XBAR DMA-transpose into [h, token] tiles,
then consumed by TensorE.  Scores come from M=1 matmuls against Wt,
softmax runs in natural layout after a K=1 transpose matmul, and the
final alpha-weighted sum runs on VectorE with per-partition scalars
against the natural-layout bf16 copy of allh.
"""

import os
import sys

sys.path.insert(0, "/opt/trn_rl_repo")

import numpy as np

import concourse.bass as bass
import concourse.mybir as mybir
import concourse.tile as tile
from concourse import bacc
from concourse.bass_utils import run_bass_kernel_spmd

F32 = mybir.dt.float32
BF16 = mybir.dt.bfloat16
AF = mybir.ActivationFunctionType
OP = mybir.AluOpType
AX = mybir.AxisListType

S = 21
H = 1024
KB = H // 128          # 8 blocks of 128 along any H-sized dim
N_CORES = 8
B_FULL = 8192
BC = B_FULL // N_CORES  # 1024 batch per core
BB = 128                # batch block (token partition dim)
NBLK = BC // BB         # 8 blocks per core
SCH = 3                 # s-values per matmul chunk
CHK = SCH * BB          # 384 moving-dim columns per matmul chunk
NCH = S // SCH          # 7 chunks
TOK = S * BB            # 2688 tokens per block


def emit(tc, io):
    nc = tc.nc
    c_d, h_d, w_d, v_d, bt_d, wt_d, y_d = (
        io["c"], io["allh"], io["W"], io["V"], io["bias_t"], io["wt"], io["y"],
    )

    ctx = tc._ctx  # ExitStack installed by build()

    consts = ctx.enter_context(tc.tile_pool(name="consts", bufs=1))
    # Weights, cast to bf16 in-flight.  Layout [128, (j, k)]: row p of block
    # j holds W[j*128+p, :].
    w_sb = consts.tile([128, KB * H], BF16)
    v_sb = consts.tile([128, KB * H], BF16)
    nc.gpsimd.dma_start(
        out=w_sb[:].rearrange("p (j k) -> p j k", j=KB),
        in_=w_d.rearrange("(j p) k -> p j k", p=128),
    )
    nc.gpsimd.dma_start(
        out=v_sb[:].rearrange("p (j k) -> p j k", j=KB),
        in_=v_d.rearrange("(j p) k -> p j k", p=128),
    )
    # bias_t: [H, S] fp32 -> [128, (kb, s)]
    bias_sb = consts.tile([128, KB * S], F32)
    nc.sync.dma_start(
        out=bias_sb[:].rearrange("p (kb s) -> p kb s", kb=KB),
        in_=bt_d.rearrange("(kb p) s -> p kb s", p=128),
    )
    # wt: [128, KB] fp32 -> bf16
    wt_sb = consts.tile([128, KB], BF16)
    nc.gpsimd.dma_start(out=wt_sb[:], in_=wt_d)
    ones1 = consts.tile([1, 1], F32)
    nc.vector.memset(ones1[:], 1.0)

    nat_pool = ctx.enter_context(tc.tile_pool(name="nat", bufs=38))
    ct_pool = ctx.enter_context(tc.tile_pool(name="ct", bufs=1))
    ht_pool = ctx.enter_context(tc.tile_pool(name="ht", bufs=9))
    wc_pool = ctx.enter_context(tc.tile_pool(name="wc", bufs=1))
    t_pool = ctx.enter_context(tc.tile_pool(name="tadd", bufs=2))
    e_pool = ctx.enter_context(tc.tile_pool(name="e", bufs=1))
    exp_pool = ctx.enter_context(tc.tile_pool(name="exp", bufs=2))
    sm_pool = ctx.enter_context(tc.tile_pool(name="sm", bufs=2))
    y_pool = ctx.enter_context(tc.tile_pool(name="y", bufs=1))

    ps_wc = ctx.enter_context(tc.tile_pool(name="ps_wc", bufs=1, space="PSUM"))
    ps_vh = ctx.enter_context(tc.tile_pool(name="ps_vh", bufs=4, space="PSUM"))
    ps_sc = ctx.enter_context(tc.tile_pool(name="ps_sc", bufs=2, space="PSUM"))
    ps_et = ctx.enter_context(tc.tile_pool(name="ps_et", bufs=1, space="PSUM"))

    for bb in range(NBLK):
        tb = bb * BB

        # ---- load natural-layout bf16 (cast in DMA), one tile per s ----
        c_nat = nat_pool.tile([128, H], BF16, tag="nat")
        nc.gpsimd.dma_start(out=c_nat[:], in_=c_d[tb : tb + BB, :])
        nat = []
        for s in range(S):
            nt = nat_pool.tile([128, H], BF16, tag="nat")
            nc.gpsimd.dma_start(out=nt[:], in_=h_d[s, tb : tb + BB, :])
            nat.append(nt)

        # ---- XBAR transposes into [h, token] layout ----
        # One [128, H] -> [128(h%128), KB(h//128), 128(tok)] transpose per
        # source tile: out[p, j, t] = in_[t, j*128 + p].
        ct = ct_pool.tile([128, KB * BB], BF16)
        nc.sync.dma_start(
            out=ct[:].rearrange("p (j t) -> p j t", j=KB),
            in_=c_nat[:],
            transpose=True,
        )
        htc = [ht_pool.tile([128, KB * CHK], BF16, tag="htc", name=f"htc{i}") for i in range(NCH)]
        for s in range(S):
            hv = htc[s // SCH][:].rearrange("p (j t) -> p j t", j=KB)
            nc.sync.dma_start(
                out=hv[:, :, (s % SCH) * BB : (s % SCH + 1) * BB],
                in_=nat[s][:],
                transpose=True,
            )

        # ---- wc = W.T @ cT for this block: [128k x 128b] per kb ----
        wc_sb = wc_pool.tile([128, KB * BB], F32)
        for kb in range(KB):
            pw = ps_wc.tile([128, BB], F32, tag="pwc")
            for j in range(KB):
                nc.tensor.matmul(
                    pw[:],
                    lhsT=w_sb[:, j * H + kb * 128 : j * H + (kb + 1) * 128],
                    rhs=ct[:, j * BB : (j + 1) * BB],
                    start=(j == 0),
                    stop=(j == KB - 1),
                )
            nc.vector.tensor_copy(wc_sb[:, kb * BB : (kb + 1) * BB], pw[:])

        # ---- main chunk loop: vh matmul, e = tanh(vh+wc+bias), scores,
        # exp, and online (unnormalized) weighted sum ----
        pet = ps_et.tile([128, S], F32)
        exp_nat = sm_pool.tile([128, S], F32, tag="expnat")
        y_num = y_pool.tile([128, H], F32, tag="ynum")
        nc.vector.memset(y_num[:], 0.0)
        for cix in range(NCH):
            s0 = cix * SCH
            t2 = t_pool.tile([128, KB * CHK], F32, tag="t2")
            e_sb = e_pool.tile([128, KB * CHK], BF16, tag="e")
            for kb in range(KB):
                pv = ps_vh.tile([128, CHK], F32, tag="pvh")
                for j in range(KB):
                    nc.tensor.matmul(
                        pv[:],
                        lhsT=v_sb[:, j * H + kb * 128 : j * H + (kb + 1) * 128],
                        rhs=htc[cix][:, j * CHK : (j + 1) * CHK],
                        start=(j == 0),
                        stop=(j == KB - 1),
                    )
                # t = vh + wc, then += bias (broadcast APs), into t2 slice
                t2v = t2[:, kb * CHK : (kb + 1) * CHK].rearrange(
                    "p (s t) -> p s t", s=SCH
                )
                wc_bc = wc_sb[:, kb * BB : (kb + 1) * BB][:, None, :].broadcast_to(
                    [128, SCH, BB]
                )
                nc.vector.tensor_tensor(
                    out=t2v,
                    in0=pv[:].rearrange("p (s t) -> p s t", s=SCH),
                    in1=wc_bc,
                    op=OP.add,
                )

            # bias add for all kb at once, then one big tanh
            t2w = t2[:].rearrange("p (kb s t) -> p kb s t", kb=KB, s=SCH)
            bias_w = bias_sb[:].rearrange("p (kb s) -> p kb s", kb=KB)[
                :, :, s0 : s0 + SCH
            ][:, :, :, None].broadcast_to([128, KB, SCH, BB])
            nc.vector.tensor_tensor(out=t2w, in0=t2w, in1=bias_w, op=OP.add)
            nc.scalar.activation(e_sb[:], t2[:], AF.Tanh)
            # scores chunk: [1, CHK] += wt_kb . e_kb
            psc = ps_sc.tile([1, CHK], F32, tag="psc")
            for kb in range(KB):
                nc.tensor.matmul(
                    psc[:],
                    lhsT=wt_sb[:, kb : kb + 1],
                    rhs=e_sb[:, kb * CHK : (kb + 1) * CHK],
                    start=(kb == 0),
                    stop=(kb == KB - 1),
                )
            exp_sb = exp_pool.tile([1, CHK], F32, tag="exp")
            nc.scalar.activation(exp_sb[:], psc[:], AF.Exp)
            # transpose this chunk's exp row to natural [b, s] (K=1 matmul),
            # evict to SBUF, and fold into the unnormalized weighted sums
            # (odd s on DVE, even s on GpSimd) right away.
            for s in range(s0, s0 + SCH):
                nc.tensor.matmul(
                    pet[:, s : s + 1],
                    lhsT=exp_sb[0:1, (s - s0) * BB : (s - s0 + 1) * BB],
                    rhs=ones1[:],
                    start=True,
                    stop=True,
                )
            nc.vector.tensor_copy(exp_nat[:, s0 : s0 + SCH], pet[:, s0 : s0 + SCH])
            for s in range(s0, s0 + SCH):
                nc.vector.scalar_tensor_tensor(
                    out=y_num[:],
                    in0=nat[s][:],
                    scalar=exp_nat[:, s : s + 1],
                    in1=y_num[:],
                    op0=OP.mult,
                    op1=OP.add,
                )

        # ---- normalize: y = y_num / sum_s exp ----
        zsum = sm_pool.tile([128, 1], F32, tag="z")
        nc.vector.reduce_sum(zsum[:], exp_nat[:], axis=AX.X)
        rz = sm_pool.tile([128, 1], F32, tag="rz")
        nc.vector.reciprocal(rz[:], zsum[:])
        nc.vector.tensor_scalar(
            out=y_num[:], in0=y_num[:], scalar1=rz[:, 0:1], scalar2=None, op0=OP.mult
        )
        nc.scalar.dma_start(out=y_d[tb : tb + BB, :], in_=y_num[:])


_CACHE = {}


def build():
    if "nc" in _CACHE:
        return _CACHE["nc"]
    import contextlib

    nc = bacc.Bacc(
        "TRN2",
        target_bir_lowering=False,
        debug=False,
        enable_asserts=False,
        num_devices=N_CORES,
    )
    io = {
        "c": nc.dram_tensor("c", [BC, H], F32, kind="ExternalInput").ap(),
        "allh": nc.dram_tensor("allh", [S, BC, H], F32, kind="ExternalInput").ap(),
        "W": nc.dram_tensor("W", [H, H], F32, kind="ExternalInput").ap(),
        "V": nc.dram_tensor("V", [H, H], F32, kind="ExternalInput").ap(),
        "bias_t": nc.dram_tensor("bias_t", [H, S], F32, kind="ExternalInput").ap(),
        "wt": nc.dram_tensor("wt", [128, KB], F32, kind="ExternalInput").ap(),
        "y": nc.dram_tensor("y", [BC, H], F32, kind="ExternalOutput").ap(),
    }
    with tile.TileContext(nc) as tc:
        with contextlib.ExitStack() as stack:
            tc._ctx = stack
            emit(tc, io)
    nc.compile()
    _CACHE["nc"] = nc
    return nc


def make_in_maps(c, allh, W, V, bias, Wt):
    c = np.asarray(c, dtype=np.float32)
    allh = np.asarray(allh, dtype=np.float32)
    W = np.asarray(W, dtype=np.float32)
    V = np.asarray(V, dtype=np.float32)
    bias_t = np.ascontiguousarray(np.asarray(bias, dtype=np.float32).T)
    wt = np.ascontiguousarray(
        np.asarray(Wt, dtype=np.float32).reshape(KB, 128).T
    )
    in_maps = []
    for i in range(N_CORES):
        sl = slice(i * BC, (i + 1) * BC)
        in_maps.append(
            {
                "c": np.ascontiguousarray(c[0, sl]),
                "allh": np.ascontiguousarray(allh[:, sl]),
                "W": W,
                "V": V,
                "bias_t": bias_t,
                "wt": wt,
            }
        )
    return in_maps


def run(in_maps, trace=False, **kw):
    nc = build()
    return run_bass_kernel_spmd(nc, in_maps, list(range(N_CORES)), trace=trace, **kw)


def kernel(c, allh, W, V, bias, Wt):
    res = run(make_in_maps(c, allh, W, V, bias, Wt))
    y = np.concatenate([res.results[i]["y"] for i in range(N_CORES)], axis=0)
    return y[None].astype(np.float32)


# revision 30
# speedup vs baseline: 1.2475x; 1.2475x over previous
"""Trainium2 Bass kernel for nn_AttentionLayer (S=21, B=8192, H=1024).

Math (per batch element b):
    wc    = c[b] @ W                       # [H]
    vh_s  = allh[s,b] @ V                  # [H] per s
    e_sb  = tanh(wc + vh_s + bias[s])      # [S, H]
    sc_sb = e_sb . Wt                      # [S]
    alpha = softmax_s(sc)                  # [S]
    y[b]  = sum_s alpha_s * allh[s,b]      # [H]

Strategy: pure data parallel over batch across 8 NeuronCores (1024 batch
each).  Per core, the V/W matmuls run in a transposed layout (H on the
partition dim).  allh is loaded fp32->bf16 (SWDGE cast DMA) in natural
layout, transposed on-chip via XBAR DMA-transpose into [h, token] tiles,
then consumed by TensorE.  Scores come from M=1 matmuls against Wt,
softmax runs in natural layout after a K=1 transpose matmul, and the
final alpha-weighted sum runs on VectorE with per-partition scalars
against the natural-layout bf16 copy of allh.
"""

import os
import sys

sys.path.insert(0, "/opt/trn_rl_repo")

import numpy as np

import concourse.bass as bass
import concourse.mybir as mybir
import concourse.tile as tile
from concourse import bacc
from concourse.bass_utils import run_bass_kernel_spmd

F32 = mybir.dt.float32
BF16 = mybir.dt.bfloat16
AF = mybir.ActivationFunctionType
OP = mybir.AluOpType
AX = mybir.AxisListType

S = 21
H = 1024
KB = H // 128          # 8 blocks of 128 along any H-sized dim
N_CORES = 8
B_FULL = 8192
BC = B_FULL // N_CORES  # 1024 batch per core
BB = 128                # batch block (token partition dim)
NBLK = BC // BB         # 8 blocks per core
SCH = 3                 # s-values per matmul chunk
CHK = SCH * BB          # 384 moving-dim columns per matmul chunk
NCH = S // SCH          # 7 chunks
TOK = S * BB            # 2688 tokens per block


def emit(tc, io):
    nc = tc.nc
    c_d, h_d, w_d, v_d, bt_d, wt_d, y_d = (
        io["c"], io["allh"], io["W"], io["V"], io["bias_t"], io["wt"], io["y"],
    )

    ctx = tc._ctx  # ExitStack installed by build()

    consts = ctx.enter_context(tc.tile_pool(name="consts", bufs=1))
    # Weights, cast to bf16 in-flight.  Layout [128, (j, k)]: row p of block
    # j holds W[j*128+p, :].
    w_sb = consts.tile([128, KB * H], BF16)
    v_sb = consts.tile([128, KB * H], BF16)
    nc.gpsimd.dma_start(
        out=w_sb[:].rearrange("p (j k) -> p j k", j=KB),
        in_=w_d.rearrange("(j p) k -> p j k", p=128),
    )
    nc.gpsimd.dma_start(
        out=v_sb[:].rearrange("p (j k) -> p j k", j=KB),
        in_=v_d.rearrange("(j p) k -> p j k", p=128),
    )
    # bias_t: [H, S] fp32 -> [128, (kb, s)]
    bias_sb = consts.tile([128, KB * S], F32)
    nc.sync.dma_start(
        out=bias_sb[:].rearrange("p (kb s) -> p kb s", kb=KB),
        in_=bt_d.rearrange("(kb p) s -> p kb s", p=128),
    )
    # wt: [128, KB] fp32 -> bf16
    wt_sb = consts.tile([128, KB], BF16)
    nc.gpsimd.dma_start(out=wt_sb[:], in_=wt_d)
    ones1 = consts.tile([1, 1], F32)
    nc.vector.memset(ones1[:], 1.0)

    nat_pool = ctx.enter_context(tc.tile_pool(name="nat", bufs=24))
    ct_pool = ctx.enter_context(tc.tile_pool(name="ct", bufs=1))
    ht_pool = ctx.enter_context(tc.tile_pool(name="ht", bufs=2))
    wc_pool = ctx.enter_context(tc.tile_pool(name="wc", bufs=1))
    t_pool = ctx.enter_context(tc.tile_pool(name="tadd", bufs=2))
    e_pool = ctx.enter_context(tc.tile_pool(name="e", bufs=1))
    exp_pool = ctx.enter_context(tc.tile_pool(name="exp", bufs=1))
    sm_pool = ctx.enter_context(tc.tile_pool(name="sm", bufs=2))
    y_pool = ctx.enter_context(tc.tile_pool(name="y", bufs=1))

    ps_wc = ctx.enter_context(tc.tile_pool(name="ps_wc", bufs=1, space="PSUM"))
    ps_vh = ctx.enter_context(tc.tile_pool(name="ps_vh", bufs=4, space="PSUM"))
    ps_sc = ctx.enter_context(tc.tile_pool(name="ps_sc", bufs=2, space="PSUM"))
    ps_et = ctx.enter_context(tc.tile_pool(name="ps_et", bufs=1, space="PSUM"))

    for bb in range(NBLK):
        tb = bb * BB

        # ---- load natural-layout bf16 (cast in DMA), one tile per s ----
        c_nat = nat_pool.tile([128, H], BF16, tag="nat")
        nc.gpsimd.dma_start(out=c_nat[:], in_=c_d[tb : tb + BB, :])
        nat = []
        for s in range(S):
            nt = nat_pool.tile([128, H], BF16, tag="nat")
            nc.gpsimd.dma_start(out=nt[:], in_=h_d[s, tb : tb + BB, :])
            nat.append(nt)

        # ---- XBAR transposes into [h, token] layout ----
        # One [128, H] -> [128(h%128), KB(h//128), 128(tok)] transpose per
        # source tile: out[p, j, t] = in_[t, j*128 + p].
        ct = ct_pool.tile([128, KB * BB], BF16)
        nc.sync.dma_start(
            out=ct[:].rearrange("p (j t) -> p j t", j=KB),
            in_=c_nat[:],
            transpose=True,
        )
        ht = ht_pool.tile([128, KB * TOK], BF16)
        ht_v = ht[:].rearrange("p (j st) -> p j st", j=KB)
        for s in range(S):
            nc.sync.dma_start(
                out=ht_v[:, :, s * BB : (s + 1) * BB],
                in_=nat[s][:],
                transpose=True,
            )

        # ---- wc = W.T @ cT for this block: [128k x 128b] per kb ----
        wc_sb = wc_pool.tile([128, KB * BB], F32)
        for kb in range(KB):
            pw = ps_wc.tile([128, BB], F32, tag="pwc")
            for j in range(KB):
                nc.tensor.matmul(
                    pw[:],
                    lhsT=w_sb[:, j * H + kb * 128 : j * H + (kb + 1) * 128],
                    rhs=ct[:, j * BB : (j + 1) * BB],
                    start=(j == 0),
                    stop=(j == KB - 1),
                )
            nc.vector.tensor_copy(wc_sb[:, kb * BB : (kb + 1) * BB], pw[:])

        # ---- main chunk loop: vh matmul, e = tanh(vh+wc+bias), scores,
        # exp, and online (unnormalized) weighted sum ----
        pet = ps_et.tile([128, S], F32)
        exp_nat = sm_pool.tile([128, S], F32, tag="expnat")
        y_num = y_pool.tile([128, H], F32, tag="ynum")
        nc.vector.memset(y_num[:], 0.0)
        for cix in range(NCH):
            s0 = cix * SCH
            t2 = t_pool.tile([128, KB * CHK], F32, tag="t2")
            e_sb = e_pool.tile([128, KB * CHK], BF16, tag="e")
            for kb in range(KB):
                pv = ps_vh.tile([128, CHK], F32, tag="pvh")
                for j in range(KB):
                    nc.tensor.matmul(
                        pv[:],
                        lhsT=v_sb[:, j * H + kb * 128 : j * H + (kb + 1) * 128],
                        rhs=ht[:, j * TOK + cix * CHK : j * TOK + (cix + 1) * CHK],
                        start=(j == 0),
                        stop=(j == KB - 1),
                    )
                # t = vh + wc, then += bias (broadcast APs), into t2 slice
                t2v = t2[:, kb * CHK : (kb + 1) * CHK].rearrange(
                    "p (s t) -> p s t", s=SCH
                )
                wc_bc = wc_sb[:, kb * BB : (kb + 1) * BB][:, None, :].broadcast_to(
                    [128, SCH, BB]
                )
                nc.vector.tensor_tensor(
                    out=t2v,
                    in0=pv[:].rearrange("p (s t) -> p s t", s=SCH),
                    in1=wc_bc,
                    op=OP.add,
                )
                b_bc = bias_sb[:, kb * S + s0 : kb * S + s0 + SCH][
                    :, :, None
                ].broadcast_to([128, SCH, BB])
                nc.vector.tensor_tensor(
                    out=t2v, in0=t2v, in1=b_bc, op=OP.add
                )
            # one big tanh over all kb of this chunk
            nc.scalar.activation(e_sb[:], t2[:], AF.Tanh)
            # scores chunk: [1, CHK] += wt_kb . e_kb
            psc = ps_sc.tile([1, CHK], F32, tag="psc")
            for kb in range(KB):
                nc.tensor.matmul(
                    psc[:],
                    lhsT=wt_sb[:, kb : kb + 1],
                    rhs=e_sb[:, kb * CHK : (kb + 1) * CHK],
                    start=(kb == 0),
                    stop=(kb == KB - 1),
                )
            exp_sb = exp_pool.tile([1, CHK], F32, tag="exp")
            nc.scalar.activation(exp_sb[:], psc[:], AF.Exp)
            # transpose this chunk's exp row to natural [b, s] (K=1 matmul),
            # evict to SBUF, and fold into the unnormalized weighted sums
            # (odd s on DVE, even s on GpSimd) right away.
            for s in range(s0, s0 + SCH):
                nc.tensor.matmul(
                    pet[:, s : s + 1],
                    lhsT=exp_sb[0:1, (s - s0) * BB : (s - s0 + 1) * BB],
                    rhs=ones1[:],
                    start=True,
                    stop=True,
                )
            nc.vector.tensor_copy(exp_nat[:, s0 : s0 + SCH], pet[:, s0 : s0 + SCH])
            for s in range(s0, s0 + SCH):
                nc.vector.scalar_tensor_tensor(
                    out=y_num[:],
                    in0=nat[s][:],
                    scalar=exp_nat[:, s : s + 1],
                    in1=y_num[:],
                    op0=OP.mult,
                    op1=OP.add,
                )

        # ---- normalize: y = y_num / sum_s exp ----
        zsum = sm_pool.tile([128, 1], F32, tag="z")
        nc.vector.reduce_sum(zsum[:], exp_nat[:], axis=AX.X)
        rz = sm_pool.tile([128, 1], F32, tag="rz")
        nc.vector.reciprocal(rz[:], zsum[:])
        nc.vector.tensor_scalar(
            out=y_num[:], in0=y_num[:], scalar1=rz[:, 0:1], scalar2=None, op0=OP.mult
        )
        nc.scalar.dma_start(out=y_d[tb : tb + BB, :], in_=y_num[:])


_CACHE = {}


def build():
    if "nc" in _CACHE:
        return _CACHE["nc"]
    import contextlib

    nc = bacc.Bacc(
        "TRN2",
        target_bir_lowering=False,
        debug=False,
        enable_asserts=False,
        num_devices=N_CORES,
    )
    io = {
        "c": nc.dram_tensor("c", [BC, H], F32, kind="ExternalInput").ap(),
        "allh": nc.dram_tensor("allh", [S, BC, H], F32, kind="ExternalInput").ap(),
        "W": nc.dram_tensor("W", [H, H], F32, kind="ExternalInput").ap(),
        "V": nc.dram_tensor("V", [H, H], F32, kind="ExternalInput").ap(),
        "bias_t": nc.dram_tensor("bias_t", [H, S], F32, kind="ExternalInput").ap(),
        "wt": nc.dram_tensor("wt", [128, KB], F32, kind="ExternalInput").ap(),
        "y": nc.dram_tensor("y", [BC, H], F32, kind="ExternalOutput").ap(),
    }
    with tile.TileContext(nc) as tc:
        with contextlib.ExitStack() as stack:
            tc._ctx = stack
            emit(tc, io)
    nc.compile()
    _CACHE["nc"] = nc
    return nc


def make_in_maps(c, allh, W, V, bias, Wt):
    c = np.asarray(c, dtype=np.float32)
    allh = np.asarray(allh, dtype=np.float32)
    W = np.asarray(W, dtype=np.float32)
    V = np.asarray(V, dtype=np.float32)
    bias_t = np.ascontiguousarray(np.asarray(bias, dtype=np.float32).T)
    wt = np.ascontiguousarray(
        np.asarray(Wt, dtype=np.float32).reshape(KB, 128).T
    )
    in_maps = []
    for i in range(N_CORES):
        sl = slice(i * BC, (i + 1) * BC)
        in_maps.append(
            {
                "c": np.ascontiguousarray(c[0, sl]),
                "allh": np.ascontiguousarray(allh[:, sl]),
                "W": W,
                "V": V,
                "bias_t": bias_t,
                "wt": wt,
            }
        )
    return in_maps


def run(in_maps, trace=False, **kw):
    nc = build()
    return run_bass_kernel_spmd(nc, in_maps, list(range(N_CORES)), trace=trace, **kw)


def kernel(c, allh, W, V, bias, Wt):
    res = run(make_in_maps(c, allh, W, V, bias, Wt))
    y = np.concatenate([res.results[i]["y"] for i in range(N_CORES)], axis=0)
    return y[None].astype(np.float32)
